# revision 1
# baseline (speedup 1.0000x reference)
"""GAU (gated attention unit, relu^2 kernel attention) on 8 TRN2 NeuronCores.

Strategy: data-parallel over batch (B=32 -> 4 per core), params replicated.
Matmuls run in fp32r (fp32 with 11-bit mantissa, 1 cycle/row on the PE for
moving dim >= 256 vs 4 cycles/row for full fp32). Weights are pre-rounded to
fp32r on the host (bit-exact with the HW rounding; verified).

Per-core, per-batch-item pipeline (N=1024 tokens, H=512, E=1024, S=128):
  P1  ScaleNorm (token-major) + PE-transpose -> xnT [h, tokens] fp32r
  P2  UV matmul: u^T feature-major [e, n] (weights stationary);
      v + base token-major [n, 1152] (xnT stationary); silu + biases
  P3  gamma/beta + rope on base (token-major), PE-transpose -> qT,kT [s, n]
  P4  scores: kT x qT -> qk^T tiles, relu (ACT) + square (DVE) -> kernelT fp32r
  P5  AV: v x kernelT -> av^T [e, n] psum; gate with u^T (DVE) -> g^T fp32r
  P6  final: g^T x Wo -> [n, h] psum; + bo + x (shortcut); DMA out
"""
import sys

for _p in ("/opt/trn_rl_repo",):
    if _p not in sys.path:
        sys.path.append(_p)

import numpy as np
import concourse.bass as bass
import concourse.mybir as mybir
import concourse.tile as tile
from concourse import bacc
from concourse.bass_utils import run_bass_kernel_spmd
from concourse.masks import make_identity

F32 = mybir.dt.float32
F32R = mybir.dt.float32r
AF = mybir.ActivationFunctionType
ALU = mybir.AluOpType

B, N, H, E, S = 32, 1024, 512, 1024, 128
NCORES = 8
BPC = B // NCORES          # batch items per core
NT = N // 128              # token tiles (8)
HC = H // 128              # hidden chunks (4)
EC = E // 128              # e chunks (8)
EPS = 1e-05
UVW = 2 * E + S            # 2176
VW = E + S                 # v+base width 1152

_CACHE = {}


def _round_f32r(a):
    """Round fp32 -> fp32r (11-bit mantissa, RNE). Bit-exact with TRN2 HW."""
    u = np.ascontiguousarray(a, dtype=np.float32).view(np.uint32).astype(np.uint64)
    r = (u + 0x7FF + ((u >> 12) & 1)) & 0xFFFFFFFF
    return (r & ~np.uint64(0xFFF)).astype(np.uint32).view(np.float32)


def _build(phases=6):
    nc = bacc.Bacc()

    x4 = nc.declare_dram_parameter("x4", [BPC, NT, 128, 512], F32, isOutput=False)
    wuv = nc.declare_dram_parameter("wuv", [128, HC * UVW], F32R, isOutput=False)
    wo = nc.declare_dram_parameter("wo", [128, EC * 512], F32R, isOutput=False)
    bu_t = nc.declare_dram_parameter("bu_t", [128, EC], F32, isOutput=False)
    bvb = nc.declare_dram_parameter("bvb", [128, VW], F32, isOutput=False)
    bsb = nc.declare_dram_parameter("bsb", [128, 1], F32, isOutput=False)
    x4b = nc.declare_dram_parameter("x4b", [BPC, NT, 128, 512], F32, isOutput=False)
    gqb = nc.declare_dram_parameter("gqb", [128, 128], F32, isOutput=False)
    bqb = nc.declare_dram_parameter("bqb", [128, 128], F32, isOutput=False)
    gkb = nc.declare_dram_parameter("gkb", [128, 128], F32, isOutput=False)
    bkb = nc.declare_dram_parameter("bkb", [128, 128], F32, isOutput=False)
    cosb = nc.declare_dram_parameter("cosb", [128, NT * 64], F32, isOutput=False)
    sinb = nc.declare_dram_parameter("sinb", [128, NT * 64], F32, isOutput=False)
    out4 = nc.declare_dram_parameter("out4", [BPC, NT, 128, 512], F32, isOutput=True)

    with tile.TileContext(nc) as tc:
        from contextlib import ExitStack

        with ExitStack() as ctx:
            const = ctx.enter_context(tc.tile_pool(name="const", bufs=1))
            wuv_t = const.tile([128, HC, UVW], F32R)
            nc.sync.dma_start(out=wuv_t, in_=wuv[:])
            wo_t = const.tile([128, EC, 512], F32R)
            nc.sync.dma_start(out=wo_t, in_=wo[:])
            but_t = const.tile([128, EC], F32)
            nc.sync.dma_start(out=but_t, in_=bu_t[:])
            bvb_t = const.tile([128, VW], F32)
            nc.sync.dma_start(out=bvb_t, in_=bvb[:])
            bsb_t = const.tile([128, 1], F32)
            nc.sync.dma_start(out=bsb_t, in_=bsb[:])
            gq_t = const.tile([128, 128], F32)
            nc.sync.dma_start(out=gq_t, in_=gqb[:])
            bq_t = const.tile([128, 128], F32)
            nc.sync.dma_start(out=bq_t, in_=bqb[:])
            gk_t = const.tile([128, 128], F32)
            nc.sync.dma_start(out=gk_t, in_=gkb[:])
            bk_t = const.tile([128, 128], F32)
            nc.sync.dma_start(out=bk_t, in_=bkb[:])
            cos_t = const.tile([128, NT, 64], F32)
            nc.sync.dma_start(out=cos_t, in_=cosb[:])
            sin_t = const.tile([128, NT, 64], F32)
            nc.sync.dma_start(out=sin_t, in_=sinb[:])
            ident = const.tile([128, 128], F32)
            make_identity(nc, ident)
            ident_r = const.tile([128, 128], F32R)
            nc.scalar.copy(ident_r, ident)

            big = ctx.enter_context(tc.tile_pool(name="big", bufs=1))
            scr = ctx.enter_context(tc.tile_pool(name="scr", bufs=2))
            sm = ctx.enter_context(tc.tile_pool(name="sm", bufs=2))
            dram = ctx.enter_context(tc.tile_pool(name="dram", bufs=1, space="DRAM"))

            def p1a_norm(b):
                """DMA x + ScaleNorm -> 8 xn tiles (DVE/ACT only, no PE/PSUM)."""
                xns = []
                with nc.named_scope(f"P1a_{b}"):
                    xts = []
                    ssall = sm.tile([128, NT], F32, tag="ssall", name=f"ssall{b}")
                    for t in range(NT):
                        x_t = sm.tile([128, 512], F32, tag="x_t", name=f"x{b}_{t}", bufs=8)
                        nc.sync.dma_start(out=x_t, in_=x4[b, t])
                        ssc = sm.tile([128, 512], F32, tag="ssc", name=f"ssc{b}_{t}", bufs=1)
                        nc.scalar.activation(ssc, x_t, AF.Square,
                                             accum_out=ssall[:, t:t + 1])
                        xts.append(x_t)
                    nrm = sm.tile([128, NT], F32, tag="nrm", name=f"nrm{b}")
                    nc.scalar.activation(nrm, ssall, AF.Sqrt, scale=1.0 / H)
                    den = sm.tile([128, NT], F32, tag="den", name=f"den{b}")
                    nc.vector.tensor_scalar_max(den, nrm, EPS)
                    inv = sm.tile([128, NT], F32, tag="inv", name=f"inv{b}")
                    nc.vector.reciprocal(inv, den)
                    for t in range(NT):
                        # in-place: x_t becomes xn
                        nc.vector.tensor_scalar_mul(xts[t], xts[t], inv[:, t:t + 1])
                        xns.append(xts[t])
                return xns

            def u_body(b, ps_u, u_d, xnT, wuv_t, but_t, sm, nc):
                for ec in range(EC):
                    pu = [ps_u.tile([128, 512], F32, tag="pu", name=f"pu{b}_{ec}_{nh}",
                                    bufs=4) for nh in range(2)]
                    for c in range(HC):
                        for nh in range(2):
                            nc.tensor.matmul(
                                pu[nh], wuv_t[:, c, ec * 128:(ec + 1) * 128],
                                xnT[:, c, nh * 512:(nh + 1) * 512],
                                start=(c == 0), stop=(c == HC - 1))
                    for nh in range(2):
                        uo = sm.tile([128, 512], F32, tag="uo", name=f"uo{b}_{ec}_{nh}")
                        nc.scalar.activation(uo, pu[nh], AF.Silu,
                                             bias=but_t[:, ec:ec + 1], scale=1.0)
                        nc.sync.dma_start(out=u_d[ec, nh], in_=uo)

            xns = p1a_norm(0)

            for b in range(BPC):
                # ---------- P1b: transpose xn -> xnT ----------
                xnT = scr.tile([128, HC, 1024], F32R, tag="scrA", name=f"xnT{b}")
                with nc.named_scope(f"P1b_{b}"), \
                     tc.tile_pool(name=f"ps_tr{b}", bufs=2, space="PSUM") as ps_tr:
                    for t in range(NT):
                        ptr = ps_tr.tile([128, 512], F32, tag="ptr", name=f"ptr{b}_{t}", bufs=4)
                        for c in range(HC):
                            nc.tensor.matmul(
                                ptr[:, c * 128:(c + 1) * 128],
                                xns[t][:, c * 128:(c + 1) * 128], ident,
                                is_transpose=True, start=(c == 0), stop=(c == HC - 1),
                                skip_group_check=True)
                        nc.scalar.copy(xnT[:, :, t * 128:(t + 1) * 128],
                                       ptr.rearrange("p (c n) -> p c n", c=HC))
                if phases < 2:
                    continue

                # ---------- P2: UV matmuls (base first, then u, then v) ----------
                u_d = dram.tile([EC, 2, 128, 512], F32, tag="u_d", name=f"u_d{b}")
                v = big.tile([128, NT, VW], F32R, tag="v", name=f"v{b}")
                with nc.named_scope(f"P2_{b}"), \
                     tc.tile_pool(name=f"ps_qk{b}", bufs=1, space="PSUM") as ps_qk, \
                     ExitStack() as pctx:
                    ps_base = pctx.enter_context(
                        tc.tile_pool(name=f"ps_base{b}", bufs=1, space="PSUM"))
                    # base first (feature-major, full-rate N=512), then transpose
                    # back to token-major so rope (DVE) can overlap the rest of P2
                    pvbs = [ps_base.tile([128, 512], F32, tag="pvb", name=f"pvb{b}_{j}",
                                         bufs=2) for j in range(2)]
                    for c in range(HC):
                        for j in range(2):
                            nc.tensor.matmul(
                                pvbs[j], wuv_t[:, c, 2 * E:2 * E + S],
                                xnT[:, c, j * 512:(j + 1) * 512],
                                start=(c == 0), stop=(c == HC - 1))
                    bT = sm.tile([128, 1024], F32R, tag="bT", name=f"bT{b}", bufs=1)
                    for j in range(2):
                        nc.scalar.activation(bT[:, j * 512:(j + 1) * 512], pvbs[j],
                                             AF.Silu, bias=bsb_t, scale=1.0)
                    for g in range(2):
                        ptb = ps_qk.tile([128, 512], F32R, tag="ptb",
                                         name=f"ptb{b}_{g}", bufs=2)
                        for i in range(4):
                            t = g * 4 + i
                            nc.tensor.matmul(
                                ptb[:, i * 128:(i + 1) * 128],
                                bT[:, t * 128:(t + 1) * 128], ident_r,
                                is_transpose=True, start=(i == 0), stop=(i == 3),
                                skip_group_check=True)
                        nc.scalar.copy(v[:, g * 4:(g + 1) * 4, E:E + S],
                                       ptb.rearrange("p (t s) -> p t s", t=4))
                    # u (feature-major), spilled to DRAM
                    pctx.close()
                    with tc.tile_pool(name=f"ps_u{b}", bufs=1, space="PSUM") as ps_u:
                        u_body(b, ps_u, u_d, xnT, wuv_t, but_t, sm, nc)
                    # v main (token-major)
                    with tc.tile_pool(name=f"ps_v{b}", bufs=1, space="PSUM") as ps_v:
                        for t in range(NT):
                            pv = [ps_v.tile([128, 512], F32, tag="pv",
                                            name=f"pv{b}_{t}_{si}", bufs=4) for si in range(2)]
                            for c in range(HC):
                                for si in range(2):
                                    nc.tensor.matmul(
                                        pv[si], xnT[:, c, t * 128:(t + 1) * 128],
                                        wuv_t[:, c, E + si * 512:E + (si + 1) * 512],
                                        start=(c == 0), stop=(c == HC - 1))
                            for si in range(2):
                                tv = sm.tile([128, 512], F32, tag=f"tv{si}", name=f"tv{b}_{t}_{si}")
                                nc.vector.tensor_add(tv, pv[si], bvb_t[:, si * 512:(si + 1) * 512])
                                nc.scalar.activation(v[:, t, si * 512:(si + 1) * 512], tv, AF.Silu)

                    if phases < 3:
                        continue

                    # ---------- P3: gamma/beta + rope (token-major) + transpose ----------
                    qT = big.tile([128, 1024], F32R, tag="qT", name=f"qT{b}")
                    kT = big.tile([128, 1024], F32R, tag="kT", name=f"kT{b}")
                    with nc.named_scope(f"P3_{b}"):
                        for (dst, g_t, be_t) in ((qT, gq_t, bq_t), (kT, gk_t, bk_t)):
                            nm = "q" if dst is qT else "k"
                            q0 = sm.tile([128, NT, 128], F32, tag="q0", name=f"{nm}0_{b}", bufs=1)
                            g_b = bass.AP(tensor=g_t.tensor, offset=g_t.offset,
                                          ap=[list(g_t.ap[0]), [0, NT], list(g_t.ap[1])])
                            b_b = bass.AP(tensor=be_t.tensor, offset=be_t.offset,
                                          ap=[list(be_t.ap[0]), [0, NT], list(be_t.ap[1])])
                            nc.vector.tensor_mul(q0, v[:, :, E:E + S], g_b)
                            nc.vector.tensor_add(q0, q0, b_b)
                            t1 = sm.tile([128, NT, 64], F32, tag="t1", name=f"{nm}t1_{b}", bufs=1)
                            t2 = sm.tile([128, NT, 64], F32, tag="t2", name=f"{nm}t2_{b}", bufs=1)
                            qro = sm.tile([128, NT, 128], F32, tag="qro", name=f"{nm}ro_{b}", bufs=1)
                            nc.vector.tensor_mul(t1, q0[:, :, 0:64], cos_t)
                            nc.vector.tensor_mul(t2, q0[:, :, 64:128], sin_t)
                            nc.vector.tensor_sub(qro[:, :, 0:64], t1, t2)
                            nc.vector.tensor_mul(t1, q0[:, :, 64:128], cos_t)
                            nc.vector.tensor_mul(t2, q0[:, :, 0:64], sin_t)
                            nc.vector.tensor_add(qro[:, :, 64:128], t1, t2)
                            for g in range(2):
                                ptq = ps_qk.tile([128, 512], F32, tag="ptq",
                                                 name=f"ptq{nm}_{b}_{g}", bufs=2)
                                for i in range(4):
                                    t = g * 4 + i
                                    nc.tensor.matmul(
                                        ptq[:, i * 128:(i + 1) * 128], qro[:, t, :], ident,
                                        is_transpose=True, start=(i == 0), stop=(i == 3),
                                        skip_group_check=True)
                                nc.scalar.copy(dst[:, g * 512:(g + 1) * 512], ptq)

                if phases < 4:
                    continue

                # ---------- P4/P5/P6 per n-half (single per-batch psum pool) ----------
                with tc.tile_pool(name=f"ps_att{b}", bufs=1, space="PSUM") as ps_att:
                    for nh in range(2):
                        kerT = scr.tile([128, NT, 512], F32R, tag="scrA", name=f"kerT{b}_{nh}")
                        with nc.named_scope(f"P4_{b}_{nh}"):
                            for m in range(NT):
                                psc = ps_att.tile([128, 512], F32, tag="pw",
                                                  name=f"psc{b}_{nh}_{m}", bufs=4)
                                nc.tensor.matmul(
                                    psc, kT[:, m * 128:(m + 1) * 128],
                                    qT[:, nh * 512:(nh + 1) * 512], start=True, stop=True)
                                rl = sm.tile([128, 512], F32, tag="rl", name=f"rl{b}_{nh}_{m}", bufs=4)
                                if m % 2 == 0:
                                    nc.scalar.activation(rl, psc, AF.Relu)
                                else:
                                    nc.vector.tensor_scalar_max(rl, psc, 0.0)
                                nc.vector.tensor_mul(kerT[:, m, :], rl, rl)
                        if phases < 5:
                            continue
                        pf = [ps_att.tile([128, 512], F32, tag=f"pf{i}",
                                          name=f"pf{b}_{nh}_{i}") for i in range(4)]
                        with nc.named_scope(f"P5_{b}_{nh}"):
                            for eg in range(EC // 2):
                                pavs = [ps_att.tile([128, 512], F32, tag="pw",
                                                    name=f"pav{b}_{nh}_{eg}_{j}", bufs=4)
                                        for j in range(2)]
                                for m in range(NT):
                                    for j in range(2):
                                        ec = eg * 2 + j
                                        nc.tensor.matmul(
                                            pavs[j], v[:, m, ec * 128:(ec + 1) * 128],
                                            kerT[:, m, :],
                                            start=(m == 0), stop=(m == NT - 1))
                                for j in range(2):
                                    ec = eg * 2 + j
                                    ui = sm.tile([128, 512], F32, tag="ui", name=f"ui{b}_{nh}_{ec}", bufs=2)
                                    nc.sync.dma_start(out=ui, in_=u_d[ec, nh])
                                    gt = sm.tile([128, 512], F32R, tag="gt", name=f"gt{b}_{nh}_{ec}", bufs=2)
                                    nc.vector.tensor_mul(gt, ui, pavs[j])
                                    for nt in range(4):
                                        nc.tensor.matmul(
                                            pf[nt], gt[:, nt * 128:(nt + 1) * 128], wo_t[:, ec, :],
                                            start=(ec == 0), stop=(ec == EC - 1))
                        if phases < 6:
                            continue
                        with nc.named_scope(f"P6_{b}_{nh}"):
                            for nt in range(4):
                                gtk = nh * 4 + nt
                                xr = sm.tile([128, 512], F32, tag="xr", name=f"xr{b}_{nh}_{nt}")
                                nc.sync.dma_start(out=xr, in_=x4b[b, gtk])
                                ot = sm.tile([128, 512], F32, tag="ot", name=f"ot{b}_{nh}_{nt}")
                                nc.vector.tensor_add(ot, pf[nt], xr)
                                nc.sync.dma_start(out=out4[b, gtk], in_=ot)
                if b + 1 < BPC:
                    xns = p1a_norm(b + 1)

    nc.finalize()
    return nc


def _host_prep(x, Wuv, buv, gamma, beta, Wo, bo, g):
    s4 = float(S) ** -0.25
    # fold g into Wuv; pre-round weights to fp32r
    wuv_f = Wuv * float(np.asarray(g).reshape(-1)[0])
    wuv_l = _round_f32r(wuv_f.reshape(HC, 128, UVW).transpose(1, 0, 2).reshape(128, HC * UVW))
    wo_l = _round_f32r(Wo.reshape(EC, 128, 512).transpose(1, 0, 2).reshape(128, EC * 512))
    bu_l = np.ascontiguousarray(buv[:E].reshape(EC, 128).T)              # [128, EC]
    bvb_l = np.broadcast_to(buv[E:], (128, VW)).copy()                   # [128, 1152]
    bsb_l = np.ascontiguousarray(buv[2 * E:].reshape(S, 1))              # [128, 1]

    gq_l = np.broadcast_to(gamma[0] * s4, (128, S)).copy()
    bq_l = np.broadcast_to(beta[0] * s4, (128, S)).copy()
    gk_l = np.broadcast_to(gamma[1] * s4, (128, S)).copy()
    bk_l = np.broadcast_to(beta[1] * s4, (128, S)).copy()
    half = S // 2
    # Tables computed via jnp with the default backend, mirroring the
    # reference's rope() exactly: sin/cos of n * 10000**(j/half) is
    # ill-conditioned in fp32 for large n*freq, so the values must come from
    # the same sin/cos implementation the reference uses in this environment.
    import jax.numpy as jnp
    pos_j = jnp.arange(N, dtype=jnp.float32)
    inv_freq_j = 10000.0 ** (jnp.arange(half, dtype=jnp.float32) / half)
    sinus_j = pos_j[:, None, None] * inv_freq_j[None, None, :]
    sin_f = np.asarray(jnp.sin(sinus_j)).reshape(N, half)
    cos_f = np.asarray(jnp.cos(sinus_j)).reshape(N, half)
    cos_l = np.ascontiguousarray(
        cos_f.reshape(NT, 128, half).transpose(1, 0, 2).reshape(128, NT * half),
        dtype=np.float32)
    sin_l = np.ascontiguousarray(
        sin_f.reshape(NT, 128, half).transpose(1, 0, 2).reshape(128, NT * half),
        dtype=np.float32)
    shared = dict(wuv=wuv_l, wo=wo_l, bu_t=bu_l.astype(np.float32),
                  bvb=bvb_l.astype(np.float32), bsb=bsb_l.astype(np.float32),
                  gqb=gq_l.astype(np.float32), bqb=bq_l.astype(np.float32),
                  gkb=gk_l.astype(np.float32), bkb=bk_l.astype(np.float32),
                  cosb=cos_l, sinb=sin_l)
    in_maps = []
    for core in range(NCORES):
        xs = np.ascontiguousarray(
            x[core * BPC:(core + 1) * BPC].reshape(BPC, NT, 128, 512), dtype=np.float32)
        xsb = xs + bo.reshape(1, 1, 1, 512).astype(np.float32)
        in_maps.append(dict(x4=xs, x4b=xsb, **shared))
    return in_maps


def kernel(x, Wuv, buv, gamma, beta, Wo, bo, g, _trace=False):
    if "nc" not in _CACHE:
        _CACHE["nc"] = _build()
    nc = _CACHE["nc"]
    in_maps = _host_prep(np.asarray(x), np.asarray(Wuv), np.asarray(buv),
                         np.asarray(gamma), np.asarray(beta), np.asarray(Wo),
                         np.asarray(bo), np.asarray(g))
    res = run_bass_kernel_spmd(nc, in_maps, list(range(NCORES)), trace=_trace)
    out = np.empty((B, N, H), dtype=np.float32)
    for core in range(NCORES):
        out[core * BPC:(core + 1) * BPC] = res.results[core]["out4"].reshape(BPC, N, H)
    if _trace:
        _CACHE["last_results"] = res
    return out



# revision 6
# speedup vs baseline: 1.3027x; 1.3027x over previous
"""GAU (gated attention unit, relu^2 kernel attention) on 8 TRN2 NeuronCores.

Data-parallel over batch (B=32 -> 4 per core), params replicated.

v2 rewrite vs baseline (528us): single fully-pipelined issue schedule,
no DRAM spill of u, feature-major rope (kills 24 transpose matmuls/item),
bf16 matmul operands everywhere except scores (f32r), per-tile norm chain,
cross-item software pipeline (next item's norm+transpose overlaps this
item's attention+output phases). PSUM budget: tag A (4 banks, transient:
transposes/scores/final) + tag B (4 banks, accumulators: uv/base/av).

Per-core, per-item (N=1024 tokens, H=512, E=1024, S=128):
  P1  x DMA -> sumsq (ACT) -> inv=1/sqrt(ss/H) -> scale (DVE, in place)
      -> PE-transpose -> xnT bf16 [h, tok]
  P2  base matmul (feat-major) + silu -> bT; affine (ACT) + rope (DVE,
      feature-major) -> qT,kT f32r; v matmul (tok-major) + bias + silu;
      u matmul (feat-major) + silu
  P45 per n-half: scores kT x qT -> psc; relu (ACT/DVE) + square (DVE)
      -> kerT bf16; AV (m outer, e-half inner) -> av^T psum; gate u*av
      -> gT bf16
  P6  final gT x Wo -> psum; + (x+bo) prefetched; DMA out
"""
import sys

for _p in ("/opt/trn_rl_repo",):
    if _p not in sys.path:
        sys.path.append(_p)

import numpy as np
import ml_dtypes
import concourse.bass as bass
import concourse.mybir as mybir
import concourse.tile as tile
from concourse import bacc
from concourse.bass_utils import run_bass_kernel_spmd
from concourse.masks import make_identity

F32 = mybir.dt.float32
F32R = mybir.dt.float32r
BF16 = mybir.dt.bfloat16
AF = mybir.ActivationFunctionType

B, N, H, E, S = 32, 1024, 512, 1024, 128
NCORES = 8
BPC = B // NCORES          # batch items per core
NT = N // 128              # token tiles (8)
HC = H // 128              # hidden chunks (4)
EC = E // 128              # e chunks (8)
UVW = 2 * E + S            # 2176
HALF = S // 2              # 64

_CACHE = {}


def _build():
    nc = bacc.Bacc()

    x4 = nc.declare_dram_parameter("x4", [BPC, NT, 128, 512], F32, isOutput=False)
    x4b = nc.declare_dram_parameter("x4b", [BPC, NT, 128, 512], F32, isOutput=False)
    wuv = nc.declare_dram_parameter("wuv", [128, HC * UVW], BF16, isOutput=False)
    wo = nc.declare_dram_parameter("wo", [128, EC * 512], BF16, isOutput=False)
    bu = nc.declare_dram_parameter("bu", [128, EC], F32, isOutput=False)
    bvb = nc.declare_dram_parameter("bvb", [128, E], F32, isOutput=False)
    bsb = nc.declare_dram_parameter("bsb", [128, 1], F32, isOutput=False)
    gqv = nc.declare_dram_parameter("gqv", [128, 1], F32, isOutput=False)
    bqv = nc.declare_dram_parameter("bqv", [128, 1], F32, isOutput=False)
    gkv = nc.declare_dram_parameter("gkv", [128, 1], F32, isOutput=False)
    bkv = nc.declare_dram_parameter("bkv", [128, 1], F32, isOutput=False)
    # rope tables duplicated across both partition halves: DVE tensor_tensor
    # requires both SBUF inputs at the same base partition.
    cosf = nc.declare_dram_parameter("cosf", [128, N], F32, isOutput=False)
    sinf = nc.declare_dram_parameter("sinf", [128, N], F32, isOutput=False)
    out4 = nc.declare_dram_parameter("out4", [BPC, NT, 128, 512], F32, isOutput=True)

    with tile.TileContext(nc) as tc:
        from contextlib import ExitStack

        with ExitStack() as ctx:
            const = ctx.enter_context(tc.tile_pool(name="const", bufs=1))
            sb = ctx.enter_context(tc.tile_pool(name="sb", bufs=1))
            ps = ctx.enter_context(tc.tile_pool(name="ps", bufs=1, space="PSUM"))

            # ---- x(0) input DMA first: the earliest PE work depends on it
            xq = {}

            def issue_x_dma(b):
                xq[b] = []
                for t in range(NT):
                    x_t = sb.tile([128, 512], F32, tag="x", name=f"x{b}_{t}", bufs=8)
                    nc.sync.dma_start(out=x_t, in_=x4[b, t])
                    xq[b].append(x_t)

            xbq = {}

            def issue_xb_dma(b):
                xbq[b] = []
                for t in range(NT):
                    x_t = sb.tile([128, 512], F32, tag="xb", name=f"xb{b}_{t}", bufs=8)
                    nc.sync.dma_start(out=x_t, in_=x4b[b, t])
                    xbq[b].append(x_t)

            issue_x_dma(0)

            # ---- constants
            wuv_t = const.tile([128, HC, UVW], BF16)
            nc.sync.dma_start(out=wuv_t, in_=wuv[:])
            bu_t = const.tile([128, EC], F32)
            nc.sync.dma_start(out=bu_t, in_=bu[:])
            bvb_t = const.tile([128, E], F32)
            nc.sync.dma_start(out=bvb_t, in_=bvb[:])
            bsb_t = const.tile([128, 1], F32)
            nc.sync.dma_start(out=bsb_t, in_=bsb[:])
            gq_t = const.tile([128, 1], F32)
            nc.sync.dma_start(out=gq_t, in_=gqv[:])
            bq_t = const.tile([128, 1], F32)
            nc.sync.dma_start(out=bq_t, in_=bqv[:])
            gk_t = const.tile([128, 1], F32)
            nc.sync.dma_start(out=gk_t, in_=gkv[:])
            bk_t = const.tile([128, 1], F32)
            nc.sync.dma_start(out=bk_t, in_=bkv[:])
            cos_t = const.tile([128, N], F32)
            nc.sync.dma_start(out=cos_t, in_=cosf[:])
            sin_t = const.tile([128, N], F32)
            nc.sync.dma_start(out=sin_t, in_=sinf[:])
            wo_t = const.tile([128, EC, 512], BF16)
            nc.sync.dma_start(out=wo_t, in_=wo[:])
            ident = const.tile([128, 128], F32)
            make_identity(nc, ident)

            # ---- P1: norm + transpose -> xnT bf16 (per-tile chain)
            def p1(b):
                xnT = sb.tile([128, HC, 1024], BF16, tag="xnT", name=f"xnT{b}",
                              bufs=2)
                with nc.named_scope(f"P1_{b}"):
                    for t in range(NT):
                        ssq = sb.tile([128, 512], F32, tag="ssq",
                                      name=f"ssq{b}_{t}", bufs=2)
                        ssc = sb.tile([128, 1], F32, tag="ssc",
                                      name=f"ssc{b}_{t}", bufs=4)
                        nc.scalar.activation(ssq, xq[b][t], AF.Square,
                                             accum_out=ssc)
                        nrm = sb.tile([128, 1], F32, tag="nrm",
                                      name=f"nrm{b}_{t}", bufs=4)
                        nc.scalar.activation(nrm, ssc, AF.Sqrt, scale=1.0 / H)
                        inv = sb.tile([128, 1], F32, tag="inv",
                                      name=f"inv{b}_{t}", bufs=4)
                        nc.vector.reciprocal(inv, nrm)
                        nc.vector.tensor_scalar_mul(xq[b][t], xq[b][t], inv)
                        ptr = ps.tile([128, 512], F32, tag="A",
                                      name=f"ptr{b}_{t}", bufs=4)
                        for c in range(HC):
                            nc.tensor.matmul(
                                ptr[:, c * 128:(c + 1) * 128],
                                xq[b][t][:, c * 128:(c + 1) * 128], ident,
                                is_transpose=True, start=(c == 0),
                                stop=(c == HC - 1), skip_group_check=True)
                        nc.scalar.copy(xnT[:, :, t * 128:(t + 1) * 128],
                                       ptr.rearrange("p (c n) -> p c n", c=HC))
                return xnT

            # ---- P3: affine + rope, feature-major (no transposes)
            def p3(b, bT):
                qT = sb.tile([128, 1024], F32R, tag="qT", name=f"qT{b}", bufs=1)
                kT = sb.tile([128, 1024], F32R, tag="kT", name=f"kT{b}", bufs=1)
                with nc.named_scope(f"P3_{b}"):
                    for (dst, gv, bv, nm) in ((qT, gq_t, bq_t, "q"),
                                              (kT, gk_t, bk_t, "k")):
                        aff = sb.tile([128, 1024], F32, tag="aff",
                                      name=f"aff{b}{nm}", bufs=2)
                        nc.scalar.activation(aff, bT, AF.Identity, bias=bv,
                                             scale=gv)
                        t1 = sb.tile([HALF, 1024], F32, tag="t1",
                                     name=f"t1{b}{nm}", bufs=2)
                        t2 = sb.tile([HALF, 1024], F32, tag="t2",
                                     name=f"t2{b}{nm}", bufs=2)
                        x1, x2 = aff[0:HALF, :], aff[HALF:128, :]
                        nc.vector.tensor_mul(t1, x1, cos_t[0:HALF, :])
                        nc.vector.tensor_mul(t2, x2, sin_t[HALF:128, :])
                        nc.vector.tensor_sub(dst[0:HALF, :], t1, t2)
                        nc.vector.tensor_mul(t1, x2, cos_t[HALF:128, :])
                        nc.vector.tensor_mul(t2, x1, sin_t[0:HALF, :])
                        nc.vector.tensor_add(dst[HALF:128, :], t1, t2)
                return qT, kT

            # ---- P2: base -> rope; v (token-major); u (feature-major)
            def p2(b, xnT):
                with nc.named_scope(f"P2b_{b}"):
                    bT = sb.tile([128, 1024], F32, tag="bT", name=f"bT{b}",
                                 bufs=2)
                    pvb = [ps.tile([128, 512], F32, tag="B",
                                   name=f"pvb{b}_{j}", bufs=4) for j in range(2)]
                    for c in range(HC):
                        for j in range(2):
                            nc.tensor.matmul(
                                pvb[j], wuv_t[:, c, 2 * E:2 * E + S],
                                xnT[:, c, j * 512:(j + 1) * 512],
                                start=(c == 0), stop=(c == HC - 1))
                    for j in range(2):
                        nc.scalar.activation(bT[:, j * 512:(j + 1) * 512],
                                             pvb[j], AF.Silu, bias=bsb_t)
                qT, kT = p3(b, bT)
                v = sb.tile([128, NT, E], BF16, tag="v", name=f"v{b}", bufs=1)
                with nc.named_scope(f"P2v_{b}"):
                    for t in range(NT):
                        pv = [ps.tile([128, 512], F32, tag="B",
                                      name=f"pv{b}_{t}_{si}", bufs=4)
                              for si in range(2)]
                        for c in range(HC):
                            for si in range(2):
                                nc.tensor.matmul(
                                    pv[si], xnT[:, c, t * 128:(t + 1) * 128],
                                    wuv_t[:, c, E + si * 512:E + (si + 1) * 512],
                                    start=(c == 0), stop=(c == HC - 1))
                        for si in range(2):
                            nc.vector.tensor_add(pv[si], pv[si],
                                                 bvb_t[:, si * 512:(si + 1) * 512])
                            nc.scalar.activation(v[:, t, si * 512:(si + 1) * 512],
                                                 pv[si], AF.Silu)
                u = sb.tile([128, EC, 1024], BF16, tag="u", name=f"u{b}", bufs=1)
                with nc.named_scope(f"P2u_{b}"):
                    for ec in range(EC):
                        pu = [ps.tile([128, 512], F32, tag="B",
                                      name=f"pu{b}_{ec}_{nh}", bufs=4)
                              for nh in range(2)]
                        for c in range(HC):
                            for nh in range(2):
                                nc.tensor.matmul(
                                    pu[nh], wuv_t[:, c, ec * 128:(ec + 1) * 128],
                                    xnT[:, c, nh * 512:(nh + 1) * 512],
                                    start=(c == 0), stop=(c == HC - 1))
                        for nh in range(2):
                            nc.scalar.activation(u[:, ec, nh * 512:(nh + 1) * 512],
                                                 pu[nh], AF.Silu,
                                                 bias=bu_t[:, ec:ec + 1])
                return v, u, qT, kT

            # ---- P4/P5 per n-half: scores -> relu^2 -> AV -> gate
            def p45(b, nh, qT, kT, v, u, ker, gt):
                with nc.named_scope(f"P4_{b}_{nh}"):
                    for m in range(NT):
                        psc = ps.tile([128, 512], F32, tag="A",
                                      name=f"psc{b}_{nh}_{m}", bufs=4)
                        nc.tensor.matmul(psc, kT[:, m * 128:(m + 1) * 128],
                                         qT[:, nh * 512:(nh + 1) * 512],
                                         start=True, stop=True)
                        rl = sb.tile([128, 512], BF16, tag="rl",
                                     name=f"rl{b}_{nh}_{m}", bufs=4)
                        if m % 2 == 0:
                            nc.scalar.activation(rl, psc, AF.Relu)
                        else:
                            nc.vector.tensor_scalar_max(rl, psc, 0.0)
                        nc.vector.tensor_mul(ker[:, m, nh * 512:(nh + 1) * 512],
                                             rl, rl)
                with nc.named_scope(f"P5_{b}_{nh}"):
                    for eh in range(2):
                        pav = [ps.tile([128, 512], F32, tag="B",
                                       name=f"pav{b}_{nh}_{eh}_{i}", bufs=4)
                               for i in range(4)]
                        for m in range(NT):
                            for i in range(4):
                                ec = eh * 4 + i
                                nc.tensor.matmul(
                                    pav[i], v[:, m, ec * 128:(ec + 1) * 128],
                                    ker[:, m, nh * 512:(nh + 1) * 512],
                                    start=(m == 0), stop=(m == NT - 1))
                        for i in range(4):
                            ec = eh * 4 + i
                            nc.vector.tensor_mul(
                                gt[:, ec, nh * 512:(nh + 1) * 512],
                                u[:, ec, nh * 512:(nh + 1) * 512], pav[i])

            # ---- P6: final matmul + residual + DMA out
            def p6(b, gt):
                with nc.named_scope(f"P6_{b}"):
                    for t in range(NT):
                        pf = ps.tile([128, 512], F32, tag="A",
                                     name=f"pf{b}_{t}", bufs=4)
                        for ec in range(EC):
                            nc.tensor.matmul(pf,
                                             gt[:, ec, t * 128:(t + 1) * 128],
                                             wo_t[:, ec, :],
                                             start=(ec == 0),
                                             stop=(ec == EC - 1))
                        ot = sb.tile([128, 512], F32, tag="ot",
                                     name=f"ot{b}_{t}", bufs=4)
                        nc.vector.tensor_add(ot, pf, xbq[b][t])
                        nc.sync.dma_start(out=out4[b, t], in_=ot)

            # ---- main pipeline
            xnT = p1(0)
            for b in range(BPC):
                if b + 1 < BPC:
                    issue_x_dma(b + 1)
                v, u, qT, kT = p2(b, xnT)
                issue_xb_dma(b)
                ker = sb.tile([128, NT, 1024], BF16, tag="ker", name=f"ker{b}",
                              bufs=1)
                gt = sb.tile([128, EC, 1024], BF16, tag="gt", name=f"gt{b}",
                             bufs=1)
                p45(b, 0, qT, kT, v, u, ker, gt)
                p45(b, 1, qT, kT, v, u, ker, gt)
                if b + 1 < BPC:
                    xnT = p1(b + 1)
                p6(b, gt)

    nc.finalize()
    return nc


def _host_prep(x, Wuv, buv, gamma, beta, Wo, bo, g):
    s4 = float(S) ** -0.25
    gscale = float(np.asarray(g).reshape(-1)[0])
    wuv_f = (Wuv * gscale).astype(np.float32)
    wuv_l = np.ascontiguousarray(
        wuv_f.reshape(HC, 128, UVW).transpose(1, 0, 2).reshape(128, HC * UVW)
    ).astype(ml_dtypes.bfloat16)
    wo_l = np.ascontiguousarray(
        Wo.reshape(EC, 128, 512).transpose(1, 0, 2).reshape(128, EC * 512)
    ).astype(ml_dtypes.bfloat16)
    bu_l = np.ascontiguousarray(buv[:E].reshape(EC, 128).T).astype(np.float32)
    bvb_l = np.broadcast_to(buv[E:2 * E], (128, E)).astype(np.float32).copy()
    bsb_l = np.ascontiguousarray(buv[2 * E:].reshape(S, 1)).astype(np.float32)

    gq_l = np.ascontiguousarray((gamma[0] * s4).reshape(S, 1)).astype(np.float32)
    bq_l = np.ascontiguousarray((beta[0] * s4).reshape(S, 1)).astype(np.float32)
    gk_l = np.ascontiguousarray((gamma[1] * s4).reshape(S, 1)).astype(np.float32)
    bk_l = np.ascontiguousarray((beta[1] * s4).reshape(S, 1)).astype(np.float32)

    # rope tables must come from the same jnp sin/cos the reference uses:
    # the angles are huge so implementation rounding dominates.
    import jax.numpy as jnp
    pos_j = jnp.arange(N, dtype=jnp.float32)
    inv_freq_j = 10000.0 ** (jnp.arange(HALF, dtype=jnp.float32) / HALF)
    sinus_j = pos_j[:, None] * inv_freq_j[None, :]
    sin_f = np.asarray(jnp.sin(sinus_j)).reshape(N, HALF)
    cos_f = np.asarray(jnp.cos(sinus_j)).reshape(N, HALF)
    cos_l = np.ascontiguousarray(np.tile(cos_f.T, (2, 1))).astype(np.float32)
    sin_l = np.ascontiguousarray(np.tile(sin_f.T, (2, 1))).astype(np.float32)

    shared = dict(wuv=wuv_l, wo=wo_l, bu=bu_l, bvb=bvb_l, bsb=bsb_l,
                  gqv=gq_l, bqv=bq_l, gkv=gk_l, bkv=bk_l,
                  cosf=cos_l, sinf=sin_l)
    in_maps = []
    for core in range(NCORES):
        xs = np.ascontiguousarray(
            x[core * BPC:(core + 1) * BPC].reshape(BPC, NT, 128, 512),
            dtype=np.float32)
        xsb = xs + bo.reshape(1, 1, 1, 512).astype(np.float32)
        in_maps.append(dict(x4=xs, x4b=xsb, **shared))
    return in_maps


def kernel(x, Wuv, buv, gamma, beta, Wo, bo, g, _trace=False):
    if "nc" not in _CACHE:
        _CACHE["nc"] = _build()
    nc = _CACHE["nc"]
    in_maps = _host_prep(np.asarray(x), np.asarray(Wuv), np.asarray(buv),
                         np.asarray(gamma), np.asarray(beta), np.asarray(Wo),
                         np.asarray(bo), np.asarray(g))
    res = run_bass_kernel_spmd(nc, in_maps, list(range(NCORES)), trace=_trace)
    out = np.empty((B, N, H), dtype=np.float32)
    for core in range(NCORES):
        out[core * BPC:(core + 1) * BPC] = res.results[core]["out4"].reshape(BPC, N, H)
    if _trace:
        _CACHE["last_results"] = res
    return out


# revision 9
# speedup vs baseline: 1.4989x; 1.1507x over previous
"""GAU (gated attention unit, relu^2 kernel attention) on 8 TRN2 NeuronCores.

Data-parallel over batch (B=32 -> 4 per core), params replicated.

v2 rewrite vs baseline (528us): single fully-pipelined issue schedule,
no DRAM spill of u, feature-major rope (kills 24 transpose matmuls/item),
bf16 matmul operands everywhere except scores (f32r), per-tile norm chain,
cross-item software pipeline (next item's norm+transpose overlaps this
item's attention+output phases). PSUM budget: tag A (4 banks, transient:
transposes/scores/final) + tag B (4 banks, accumulators: uv/base/av).

Per-core, per-item (N=1024 tokens, H=512, E=1024, S=128):
  P1  x DMA -> sumsq (ACT) -> inv=1/sqrt(ss/H) -> scale (DVE, in place)
      -> PE-transpose -> xnT bf16 [h, tok]
  P2  base matmul (feat-major) + silu -> bT; affine (ACT) + rope (DVE,
      feature-major) -> qT,kT f32r; v matmul (tok-major) + bias + silu;
      u matmul (feat-major) + silu
  P45 per n-half: scores kT x qT -> psc; relu (ACT/DVE) + square (DVE)
      -> kerT bf16; AV (m outer, e-half inner) -> av^T psum; gate u*av
      -> gT bf16
  P6  final gT x Wo -> psum; + (x+bo) prefetched; DMA out
"""
import sys

for _p in ("/opt/trn_rl_repo",):
    if _p not in sys.path:
        sys.path.append(_p)

import numpy as np
import ml_dtypes
import concourse.bass as bass
import concourse.mybir as mybir
import concourse.tile as tile
from concourse import bacc
from concourse.bass_utils import run_bass_kernel_spmd
from concourse.masks import make_identity

F32 = mybir.dt.float32
F32R = mybir.dt.float32r
BF16 = mybir.dt.bfloat16
AF = mybir.ActivationFunctionType

B, N, H, E, S = 32, 1024, 512, 1024, 128
NCORES = 8
BPC = B // NCORES          # batch items per core
NT = N // 128              # token tiles (8)
HC = H // 128              # hidden chunks (4)
EC = E // 128              # e chunks (8)
UVW = 2 * E + S            # 2176
HALF = S // 2              # 64

_CACHE = {}


def _build():
    nc = bacc.Bacc()

    x4 = nc.declare_dram_parameter("x4", [BPC, NT, 128, 512], F32, isOutput=False)
    x4b = nc.declare_dram_parameter("x4b", [BPC, NT, 128, 512], F32, isOutput=False)
    wuv = nc.declare_dram_parameter("wuv", [128, HC * UVW], BF16, isOutput=False)
    wo = nc.declare_dram_parameter("wo", [128, EC * 512], BF16, isOutput=False)
    bu = nc.declare_dram_parameter("bu", [128, EC], F32, isOutput=False)
    bvb = nc.declare_dram_parameter("bvb", [128, E], F32, isOutput=False)
    bsb = nc.declare_dram_parameter("bsb", [128, 1], F32, isOutput=False)
    gqv = nc.declare_dram_parameter("gqv", [128, 1], F32, isOutput=False)
    bqv = nc.declare_dram_parameter("bqv", [128, 1], F32, isOutput=False)
    gkv = nc.declare_dram_parameter("gkv", [128, 1], F32, isOutput=False)
    bkv = nc.declare_dram_parameter("bkv", [128, 1], F32, isOutput=False)
    # rope tables duplicated across both partition halves: DVE tensor_tensor
    # requires both SBUF inputs at the same base partition.
    cosf = nc.declare_dram_parameter("cosf", [128, N], BF16, isOutput=False)
    sinf = nc.declare_dram_parameter("sinf", [128, N], BF16, isOutput=False)
    out4 = nc.declare_dram_parameter("out4", [BPC, NT, 128, 512], F32, isOutput=True)

    with tile.TileContext(nc) as tc:
        from contextlib import ExitStack

        with ExitStack() as ctx:
            const = ctx.enter_context(tc.tile_pool(name="const", bufs=1))
            sb = ctx.enter_context(tc.tile_pool(name="sb", bufs=1))
            ps = ctx.enter_context(tc.tile_pool(name="ps", bufs=1, space="PSUM"))

            # ---- x(0) input DMA first: the earliest PE work depends on it
            xq = {}

            def issue_x_dma(b):
                xq[b] = []
                for t in range(NT):
                    x_t = sb.tile([128, 512], F32, tag="x", name=f"x{b}_{t}", bufs=8)
                    nc.sync.dma_start(out=x_t, in_=x4[b, t])
                    xq[b].append(x_t)

            xbq = {}

            def issue_xb_dma(b):
                xbq[b] = []
                for t in range(NT):
                    x_t = sb.tile([128, 512], F32, tag="xb", name=f"xb{b}_{t}", bufs=8)
                    nc.sync.dma_start(out=x_t, in_=x4b[b, t])
                    xbq[b].append(x_t)

            issue_x_dma(0)

            # ---- constants
            wuv_t = const.tile([128, HC, UVW], BF16)
            nc.sync.dma_start(out=wuv_t, in_=wuv[:])
            bu_t = const.tile([128, EC], F32)
            nc.sync.dma_start(out=bu_t, in_=bu[:])
            bvb_t = const.tile([128, E], F32)
            nc.sync.dma_start(out=bvb_t, in_=bvb[:])
            bsb_t = const.tile([128, 1], F32)
            nc.sync.dma_start(out=bsb_t, in_=bsb[:])
            gq_t = const.tile([128, 1], F32)
            nc.sync.dma_start(out=gq_t, in_=gqv[:])
            bq_t = const.tile([128, 1], F32)
            nc.sync.dma_start(out=bq_t, in_=bqv[:])
            gk_t = const.tile([128, 1], F32)
            nc.sync.dma_start(out=gk_t, in_=gkv[:])
            bk_t = const.tile([128, 1], F32)
            nc.sync.dma_start(out=bk_t, in_=bkv[:])
            cos_t = const.tile([128, N], BF16)
            nc.sync.dma_start(out=cos_t, in_=cosf[:])
            sin_t = const.tile([128, N], BF16)
            nc.sync.dma_start(out=sin_t, in_=sinf[:])
            wo_t = const.tile([128, EC, 512], BF16)
            nc.sync.dma_start(out=wo_t, in_=wo[:])
            ident = const.tile([128, 128], F32)
            make_identity(nc, ident)

            # ---- P1: norm + transpose -> xnT bf16 (per-tile chain)
            def p1(b):
                xnT = sb.tile([128, HC, 1024], BF16, tag="xnT", name=f"xnT{b}",
                              bufs=2)
                with nc.named_scope(f"P1_{b}"):
                    for t in range(NT):
                        ssq = sb.tile([128, 512], F32, tag="ssq",
                                      name=f"ssq{b}_{t}", bufs=2)
                        ssc = sb.tile([128, 1], F32, tag="ssc",
                                      name=f"ssc{b}_{t}", bufs=4)
                        nc.scalar.activation(ssq, xq[b][t], AF.Square,
                                             accum_out=ssc)
                        nrm = sb.tile([128, 1], F32, tag="nrm",
                                      name=f"nrm{b}_{t}", bufs=4)
                        nc.scalar.activation(nrm, ssc, AF.Sqrt, scale=1.0 / H)
                        inv = sb.tile([128, 1], F32, tag="inv",
                                      name=f"inv{b}_{t}", bufs=4)
                        nc.vector.reciprocal(inv, nrm)
                        nc.vector.tensor_scalar_mul(xq[b][t], xq[b][t], inv)
                        ptr = ps.tile([128, 512], F32, tag="A",
                                      name=f"ptr{b}_{t}", bufs=4)
                        for c in range(HC):
                            nc.tensor.matmul(
                                ptr[:, c * 128:(c + 1) * 128],
                                xq[b][t][:, c * 128:(c + 1) * 128], ident,
                                is_transpose=True, start=(c == 0),
                                stop=(c == HC - 1), skip_group_check=True)
                        nc.scalar.copy(xnT[:, :, t * 128:(t + 1) * 128],
                                       ptr.rearrange("p (c n) -> p c n", c=HC))
                return xnT

            # ---- P3: affine + rope, feature-major (no transposes)
            def p3(b, bT):
                qT = sb.tile([128, 1024], BF16, tag="qT", name=f"qT{b}", bufs=1)
                kT = sb.tile([128, 1024], BF16, tag="kT", name=f"kT{b}", bufs=1)
                with nc.named_scope(f"P3_{b}"):
                    for (dst, gv, bv, nm) in ((qT, gq_t, bq_t, "q"),
                                              (kT, gk_t, bk_t, "k")):
                        aff = sb.tile([128, 1024], BF16, tag="aff",
                                      name=f"aff{b}{nm}", bufs=2)
                        nc.scalar.activation(aff, bT, AF.Identity, bias=bv,
                                             scale=gv)
                        t1 = sb.tile([HALF, 1024], BF16, tag="t1",
                                     name=f"t1{b}{nm}", bufs=2)
                        t2 = sb.tile([HALF, 1024], BF16, tag="t2",
                                     name=f"t2{b}{nm}", bufs=2)
                        x1, x2 = aff[0:HALF, :], aff[HALF:128, :]
                        nc.vector.tensor_mul(t1, x1, cos_t[0:HALF, :])
                        nc.vector.tensor_mul(t2, x2, sin_t[HALF:128, :])
                        nc.vector.tensor_sub(dst[0:HALF, :], t1, t2)
                        nc.vector.tensor_mul(t1, x2, cos_t[HALF:128, :])
                        nc.vector.tensor_mul(t2, x1, sin_t[0:HALF, :])
                        nc.vector.tensor_add(dst[HALF:128, :], t1, t2)
                return qT, kT

            # ---- P2: base -> rope; v (token-major); u (feature-major)
            def p2(b, xnT):
                with nc.named_scope(f"P2b_{b}"):
                    bT = sb.tile([128, 1024], F32, tag="bT", name=f"bT{b}",
                                 bufs=2)
                    pvb = [ps.tile([128, 512], F32, tag="B",
                                   name=f"pvb{b}_{j}", bufs=4) for j in range(2)]
                    for c in range(HC):
                        for j in range(2):
                            nc.tensor.matmul(
                                pvb[j], wuv_t[:, c, 2 * E:2 * E + S],
                                xnT[:, c, j * 512:(j + 1) * 512],
                                start=(c == 0), stop=(c == HC - 1))
                    for j in range(2):
                        nc.scalar.activation(bT[:, j * 512:(j + 1) * 512],
                                             pvb[j], AF.Silu, bias=bsb_t)
                v = sb.tile([128, NT, E], BF16, tag="v", name=f"v{b}", bufs=1)
                with nc.named_scope(f"P2v_{b}"):
                    for t in range(NT):
                        pv = [ps.tile([128, 512], F32, tag="B",
                                      name=f"pv{b}_{t}_{si}", bufs=4)
                              for si in range(2)]
                        for c in range(HC):
                            for si in range(2):
                                nc.tensor.matmul(
                                    pv[si], xnT[:, c, t * 128:(t + 1) * 128],
                                    wuv_t[:, c, E + si * 512:E + (si + 1) * 512],
                                    start=(c == 0), stop=(c == HC - 1))
                        for si in range(2):
                            nc.vector.tensor_add(pv[si], pv[si],
                                                 bvb_t[:, si * 512:(si + 1) * 512])
                            nc.scalar.activation(v[:, t, si * 512:(si + 1) * 512],
                                                 pv[si], AF.Silu)
                # rope issued AFTER the v-loop: its 12 DVE ops must not queue
                # ahead of the v bias-adds (pv psum recycling stalls the PE).
                qT, kT = p3(b, bT)
                u = sb.tile([128, EC, 1024], BF16, tag="u", name=f"u{b}", bufs=1)
                with nc.named_scope(f"P2u_{b}"):
                    for ec in range(EC):
                        pu = [ps.tile([128, 512], F32, tag="B",
                                      name=f"pu{b}_{ec}_{nh}", bufs=4)
                              for nh in range(2)]
                        for c in range(HC):
                            for nh in range(2):
                                nc.tensor.matmul(
                                    pu[nh], wuv_t[:, c, ec * 128:(ec + 1) * 128],
                                    xnT[:, c, nh * 512:(nh + 1) * 512],
                                    start=(c == 0), stop=(c == HC - 1))
                        for nh in range(2):
                            nc.scalar.activation(u[:, ec, nh * 512:(nh + 1) * 512],
                                                 pu[nh], AF.Silu,
                                                 bias=bu_t[:, ec:ec + 1])
                return v, u, qT, kT

            # ---- P4/P5 per n-half: scores -> relu^2 -> AV -> gate
            def p45(b, nh, qT, kT, v, u, ker, gt):
                with nc.named_scope(f"P4_{b}_{nh}"):
                    for m in range(NT):
                        psc = ps.tile([128, 512], F32, tag="A",
                                      name=f"psc{b}_{nh}_{m}", bufs=4)
                        nc.tensor.matmul(psc, kT[:, m * 128:(m + 1) * 128],
                                         qT[:, nh * 512:(nh + 1) * 512],
                                         start=True, stop=True)
                        rl = sb.tile([128, 512], BF16, tag="rl",
                                     name=f"rl{b}_{nh}_{m}", bufs=4)
                        if m % 2 == 0:
                            nc.scalar.activation(rl, psc, AF.Relu)
                        else:
                            nc.vector.tensor_scalar_max(rl, psc, 0.0)
                        nc.vector.tensor_mul(ker[:, m, nh * 512:(nh + 1) * 512],
                                             rl, rl)
                with nc.named_scope(f"P5_{b}_{nh}"):
                    for eh in range(2):
                        pav = [ps.tile([128, 512], F32, tag="B",
                                       name=f"pav{b}_{nh}_{eh}_{i}", bufs=4)
                               for i in range(4)]
                        for m in range(NT):
                            for i in range(4):
                                ec = eh * 4 + i
                                nc.tensor.matmul(
                                    pav[i], v[:, m, ec * 128:(ec + 1) * 128],
                                    ker[:, m, nh * 512:(nh + 1) * 512],
                                    start=(m == 0), stop=(m == NT - 1))
                        for i in range(4):
                            ec = eh * 4 + i
                            nc.vector.tensor_mul(
                                gt[:, ec, nh * 512:(nh + 1) * 512],
                                u[:, ec, nh * 512:(nh + 1) * 512], pav[i])

            # ---- P6: final matmul + residual + DMA out
            def p6(b, gt):
                with nc.named_scope(f"P6_{b}"):
                    for t in range(NT):
                        pf = ps.tile([128, 512], F32, tag="A",
                                     name=f"pf{b}_{t}", bufs=4)
                        for ec in range(EC):
                            nc.tensor.matmul(pf,
                                             gt[:, ec, t * 128:(t + 1) * 128],
                                             wo_t[:, ec, :],
                                             start=(ec == 0),
                                             stop=(ec == EC - 1))
                        ot = sb.tile([128, 512], F32, tag="ot",
                                     name=f"ot{b}_{t}", bufs=4)
                        nc.vector.tensor_add(ot, pf, xbq[b][t])
                        nc.sync.dma_start(out=out4[b, t], in_=ot)

            # ---- main pipeline
            xnT = p1(0)
            for b in range(BPC):
                if b + 1 < BPC:
                    issue_x_dma(b + 1)
                v, u, qT, kT = p2(b, xnT)
                issue_xb_dma(b)
                ker = sb.tile([128, NT, 1024], BF16, tag="ker", name=f"ker{b}",
                              bufs=1)
                gt = sb.tile([128, EC, 1024], BF16, tag="gt", name=f"gt{b}",
                             bufs=1)
                p45(b, 0, qT, kT, v, u, ker, gt)
                p45(b, 1, qT, kT, v, u, ker, gt)
                if b + 1 < BPC:
                    xnT = p1(b + 1)
                p6(b, gt)

    nc.finalize()
    return nc


def _host_prep(x, Wuv, buv, gamma, beta, Wo, bo, g):
    s4 = float(S) ** -0.25
    gscale = float(np.asarray(g).reshape(-1)[0])
    wuv_f = (Wuv * gscale).astype(np.float32)
    wuv_l = np.ascontiguousarray(
        wuv_f.reshape(HC, 128, UVW).transpose(1, 0, 2).reshape(128, HC * UVW)
    ).astype(ml_dtypes.bfloat16)
    wo_l = np.ascontiguousarray(
        Wo.reshape(EC, 128, 512).transpose(1, 0, 2).reshape(128, EC * 512)
    ).astype(ml_dtypes.bfloat16)
    bu_l = np.ascontiguousarray(buv[:E].reshape(EC, 128).T).astype(np.float32)
    bvb_l = np.broadcast_to(buv[E:2 * E], (128, E)).astype(np.float32).copy()
    bsb_l = np.ascontiguousarray(buv[2 * E:].reshape(S, 1)).astype(np.float32)

    gq_l = np.ascontiguousarray((gamma[0] * s4).reshape(S, 1)).astype(np.float32)
    bq_l = np.ascontiguousarray((beta[0] * s4).reshape(S, 1)).astype(np.float32)
    gk_l = np.ascontiguousarray((gamma[1] * s4).reshape(S, 1)).astype(np.float32)
    bk_l = np.ascontiguousarray((beta[1] * s4).reshape(S, 1)).astype(np.float32)

    # rope tables must come from the same jnp sin/cos the reference uses:
    # the angles are huge so implementation rounding dominates.
    import jax.numpy as jnp
    pos_j = jnp.arange(N, dtype=jnp.float32)
    inv_freq_j = 10000.0 ** (jnp.arange(HALF, dtype=jnp.float32) / HALF)
    sinus_j = pos_j[:, None] * inv_freq_j[None, :]
    sin_f = np.asarray(jnp.sin(sinus_j)).reshape(N, HALF)
    cos_f = np.asarray(jnp.cos(sinus_j)).reshape(N, HALF)
    cos_l = np.ascontiguousarray(np.tile(cos_f.T, (2, 1))).astype(ml_dtypes.bfloat16)
    sin_l = np.ascontiguousarray(np.tile(sin_f.T, (2, 1))).astype(ml_dtypes.bfloat16)

    shared = dict(wuv=wuv_l, wo=wo_l, bu=bu_l, bvb=bvb_l, bsb=bsb_l,
                  gqv=gq_l, bqv=bq_l, gkv=gk_l, bkv=bk_l,
                  cosf=cos_l, sinf=sin_l)
    in_maps = []
    for core in range(NCORES):
        xs = np.ascontiguousarray(
            x[core * BPC:(core + 1) * BPC].reshape(BPC, NT, 128, 512),
            dtype=np.float32)
        xsb = xs + bo.reshape(1, 1, 1, 512).astype(np.float32)
        in_maps.append(dict(x4=xs, x4b=xsb, **shared))
    return in_maps


def kernel(x, Wuv, buv, gamma, beta, Wo, bo, g, _trace=False):
    if "nc" not in _CACHE:
        _CACHE["nc"] = _build()
    nc = _CACHE["nc"]
    in_maps = _host_prep(np.asarray(x), np.asarray(Wuv), np.asarray(buv),
                         np.asarray(gamma), np.asarray(beta), np.asarray(Wo),
                         np.asarray(bo), np.asarray(g))
    res = run_bass_kernel_spmd(nc, in_maps, list(range(NCORES)), trace=_trace)
    out = np.empty((B, N, H), dtype=np.float32)
    for core in range(NCORES):
        out[core * BPC:(core + 1) * BPC] = res.results[core]["out4"].reshape(BPC, N, H)
    if _trace:
        _CACHE["last_results"] = res
    return out
